# revision 42
# baseline (speedup 1.0000x reference)
"""2-layer GCN (PyG GCNConv semantics) on 8 Trainium2 NeuronCores.

Math: out = A_hat @ relu(A_hat @ X @ W1 + b1) @ W2 + b2,
      A_hat = D^-1/2 (A + I) D^-1/2, D = in-degree + 1.

dinv is folded into the tables on the producer side (linearity):
  g1 = (dinv*x) @ W1;  acc[c] = sum_{e:src->c} g1[src]  (incl. self-loop);
  z  = dinv * relu(dinv*acc1 + b1);  g2 = z @ W2;
  out[c] = dinv[c]*acc2[c] + b2.

Sharding: destination-node ranges (12500 nodes/core). Each core:
  Phase A: compute the FULL g1 table (fp16) redundantly, one group at a
           time into per-group DRAM tensors (lets group-g gathers start
           while group g+1 is still being built).
  Phase B: batched dma_gather of edge-source rows (edges laid out in uniform
           (source-group, dest-window) cells so one SPMD program serves all
           cores), segment-sum via one-hot is_equal + fp16 matmul into PSUM
           per 128-dest window, accumulated into an fp32 SBUF accumulator.
  Phase C: finalize layer 1 per window, compute g2 shard (fp16, padded to
           128 cols so the layer-2 gather elem stays a 256B multiple).
  AllGather g2 shards -> full g2 table.
  Phase D: layer-2 aggregation (same machinery).
  Phase E: finalize layer 2, write output shard.

The int16 gather-index limit (32767) forces grouping edges by 25088-row
source ranges; trailing chunk padding inside each (window,group) cell uses
src_local=0 (harmless re-read of row 0) and dest_local=-1 (one-hot all-zero).

SWDGE tuning: 4 software-DGE queues round-robined across gather calls and a
32KB/partition descriptor ring carveout -- desc-gen on the Q7 otherwise
stalls on ring space and throttles the whole aggregation.
"""

import contextlib
import os
import numpy as np

import concourse.bass as bass
from concourse import bacc
import concourse.mybir as mybir
import concourse.tile as tile
from concourse import bass_utils

F32 = mybir.dt.float32
F16 = mybir.dt.float16
I16 = mybir.dt.int16

NCORES = 8
NGROUPS = 4
CALL = int(os.environ.get('GCN_CALL', '8192'))   # gather-call size in slots
NQUEUES = int(os.environ.get('GCN_NQUEUES', '4'))
DMA_SCRATCH = int(os.environ.get('GCN_DMA_SCRATCH', '32768'))
PHA_WIN = 8          # windows per phase-A iteration
MSG_BUFS = int(os.environ.get('GCN_MSG_BUFS', '3'))


def _cfg(n_nodes, f1, f2):
    shard = n_nodes // NCORES
    nw = (shard + 127) // 128
    shard_pad = nw * 128
    rows = NCORES * shard_pad          # padded table rows (both layers)
    gw = -(-rows // NGROUPS)
    gw = ((gw + 127) // 128) * 128     # group width, multiple of 128
    assert gw <= 32768, gw
    return dict(N=n_nodes, F1=f1, F2=f2, SHARD=shard, NW=nw,
                SHARD_PAD=shard_pad, ROWS=rows, GW=gw,
                XW=rows // 128)


def _layout(cnt_kwg, nw):
    """cnt_kwg: [NCORES, NW, NGROUPS] edge counts. Returns cell capacities
    C[g][w] (multiples of 64 -- halves the roundup padding, and padding is
    pure Q7 descriptor-generation waste), cell slot offsets, total slots S,
    gather call list [(g, slot0, nslots)] (128-aligned), and the per-64-slot
    half-chunk -> cell map (-1 for group-alignment filler)."""
    cmax = cnt_kwg.max(axis=0)                      # [NW, NGROUPS]
    C = ((cmax + 63) // 64) * 64                    # [NW, NGROUPS]
    cell_off = np.zeros((NGROUPS, nw), np.int64)
    off = 0
    calls = []
    cells = []            # list of (g, w, hc0, nhc) in 64-slot units
    hc_cell = []          # per global half-chunk -> cell index (-1 filler)
    for g in range(NGROUPS):
        g0 = off
        for w in range(nw):
            cell_off[g, w] = off
            c = int(C[w, g])
            if c > 0:
                ci = len(cells)
                cells.append((g, w, off // 64, c // 64))
                hc_cell.extend([ci] * (c // 64))
            off += c
        if (off - g0) % 128:
            hc_cell.append(-1)    # keep groups (and thus calls) 128-aligned
            off += 64
        s = g0
        while s < off:
            ns = min(CALL, off - s)
            calls.append((g, s, ns))
            s += ns
        assert (off - g0) % 128 == 0
    return C, cell_off, off, calls, cells, np.asarray(hc_cell)


def _pack_edges(slot, lidx, dl, S):
    """Build the wrapped int16 index tensor and packed dest-local tensor.

    Padding slots gather row 0 of the group (harmless) and carry dst=-1 so
    the one-hot drops their message."""
    src = np.zeros(S, np.int16)
    dst = np.full(S, -1.0, np.float16)
    src[slot] = lidx.astype(np.int16)
    dst[slot] = dl.astype(np.float16)
    idx_w = np.tile(src.reshape(S // 16, 16).T, (8, 1)).copy()    # [128, S/16]
    dst_p = dst.reshape(S // 128, 128).T.copy()                   # [128, S/128]
    return idx_w, dst_p


def _prep(x, edge_index, W1, b1, W2, b2, cfg):
    N, SHARD, NW, SHARD_PAD, GW = (cfg["N"], cfg["SHARD"], cfg["NW"],
                                   cfg["SHARD_PAD"], cfg["GW"])
    ROWS, F1, F2, XW = cfg["ROWS"], cfg["F1"], cfg["F2"], cfg["XW"]

    row = np.asarray(edge_index[0]).astype(np.int64)
    col = np.asarray(edge_index[1]).astype(np.int64)
    deg = (np.bincount(col, minlength=N) + 1).astype(np.float32)
    dinv = (1.0 / np.sqrt(deg)).astype(np.float32)
    # self-loops bypass the gather entirely: their message for dest c is just
    # the table row c, added straight into the accumulator on-device (the
    # per-core own-shard x slice arrives as a dedicated input).

    k_arr = col // SHARD
    rel = col - k_arr * SHARD
    wl = rel >> 7
    dl = rel & 127

    # layer-1 source ids: raw. layer-2: adjusted to padded-shard table rows.
    src1 = row
    src2 = (row // SHARD) * SHARD_PAD + (row % SHARD)

    layers = []
    for src in (src1, src2):
        g = src // GW
        lidx = src - g * GW
        key = (k_arr * NW + wl) * NGROUPS + g
        cnt = np.bincount(key, minlength=NCORES * NW * NGROUPS)
        cnt = cnt.reshape(NCORES, NW, NGROUPS)
        C, cell_off, S, calls, cells, hc_cell = _layout(cnt, NW)
        order = np.lexsort((wl, g, k_arr))
        ks, gs, ws = k_arr[order], g[order], wl[order]
        keys = (ks * NGROUPS + gs) * NW + ws
        starts = np.r_[0, np.nonzero(np.diff(keys))[0] + 1]
        run_id = np.zeros(len(keys), np.int64)
        run_id[starts[1:]] = 1
        run_id = np.cumsum(run_id)
        within = np.arange(len(keys)) - starts[run_id]
        slot = cell_off[gs, ws] + within
        idx_list, dst_list = [], []
        core_slots = []
        for k in range(NCORES):
            m = ks == k
            iw, dp = _pack_edges(slot[m], lidx[order][m], dl[order][m], S)
            idx_list.append(iw)
            dst_list.append(dp)
            core_slots.append(np.sort(slot[m]))
        # per call: first chunk that may hold trailing-trimmed (never-written)
        # slots on SOME core -- the Q7 skips trailing idx=-1 slots, so those
        # message bytes stay whatever was in SBUF unless we pre-memset them.
        trim0 = []
        for (g_, s0, ns) in calls:
            mstart = s0 + ns
            for cs in core_slots:
                j = np.searchsorted(cs, s0 + ns)
                last = cs[j - 1] if j > 0 else -1
                start = last + 1 if last >= s0 else s0
                mstart = min(mstart, start)
            trim0.append(max(0, (mstart - s0) // 128))
        layers.append(dict(S=S, calls=calls, cells=cells, trim0=trim0,
                           hc_cell=hc_cell, idx=idx_list, dst=dst_list))

    # dinv folded into x on the host: g1 = (dinv*x) @ W1
    xs = np.asarray(x, np.float32) * dinv[:, None]
    xT = np.zeros((F1, ROWS), np.float16)
    xT[:, :N] = xs.T.astype(np.float16)
    # per-core own-shard slice of (dinv*x).T -- feeds the self-loop term
    xTo = []
    for k in range(NCORES):
        o = np.zeros((F1, SHARD_PAD), np.float16)
        o[:, :SHARD] = xs[k * SHARD:(k + 1) * SHARD].T.astype(np.float16)
        xTo.append(o)
    dinv_ext = np.zeros(ROWS, np.float32)
    dinv_ext[:N] = dinv
    dinvO = [dinv_ext[k * SHARD:k * SHARD + SHARD_PAD].reshape(NW, 128).T.copy()
             for k in range(NCORES)]
    iota = np.tile(np.arange(128, dtype=np.float16), (128, 1))
    ident = np.eye(128, dtype=np.float16)
    b1b = np.tile(np.asarray(b1, np.float32), (128, 1))
    b2b = np.tile(np.asarray(b2, np.float32), (128, 1))

    in_maps = []
    for k in range(NCORES):
        in_maps.append({
            "xT": xT, "xTo": xTo[k],
            "W1": np.asarray(W1, np.float16),
            "W2": np.asarray(W2, np.float16),
            "b1b": b1b, "b2b": b2b,
            "dinvO": dinvO[k], "iota": iota, "ident": ident,
            "idx1": layers[0]["idx"][k], "dst1": layers[0]["dst"][k],
            "idx2": layers[1]["idx"][k], "dst2": layers[1]["dst"][k],
        })
    meta = dict(L1=layers[0], L2=layers[1])
    return in_maps, meta


def _emit_agg_group(nc, meta_l, table, elem, used, acc, iota_sb, pools,
                    g_only, preloaded={}):
    """One source-group's gather calls + one-hot matmuls + SBUF accumulate.

    table["tbl"][g] is the per-group DRAM table (fp16, `elem` cols of which
    the first `used` are real data).
    """
    calls, cells, hc_cell = meta_l["calls"], meta_l["cells"], meta_l["hc_cell"]
    idx_d = table["idx"]
    dst_d = table["dst"]
    tbls = table["tbl"]
    sb2, ixp, ohp, psp = pools
    if True:
        cell_psum = {}
        cell_done = {}
        for ci, (g, s0, ns) in enumerate(calls):
            if g != g_only:
                continue
            nch = ns // 128
            # idx/dst prefetch on the Activation HWDGE queue, deep pool --
            # keeps the gather's desc-gen fed without waiting on the Sync
            # engine's DMA backlog or the msg-tile rotation
            if ci in preloaded:
                idx_t, dst_t = preloaded[ci]
            else:
                idx_t = ixp.tile([128, CALL // 16], I16, tag="idx")
                nc.scalar.dma_start(out=idx_t[:, :ns // 16],
                                    in_=idx_d[:, s0 // 16:(s0 + ns) // 16])
                dst_t = ixp.tile([128, CALL // 128], F16, tag="dst")
                nc.scalar.dma_start(out=dst_t[:, :nch],
                                    in_=dst_d[:, s0 // 128:(s0 + ns) // 128])
            msg_t = sb2.tile([128, CALL // 128, elem], F16, tag="msg")
            nc.gpsimd.dma_gather(
                msg_t[:, :nch, :], tbls[g][:, :],
                idx_t[:, :ns // 16], ns, ns, elem, elem_step=elem,
                queue_num=ci % NQUEUES, single_packet=False,
            )
            oh_tiles = []
            for h in range(0, nch, 32):
                hn = min(32, nch - h)
                oh = ohp.tile([128, 32, 128], F16, tag="oh")
                nc.vector.tensor_tensor(
                    out=oh[:, :hn, :],
                    in0=dst_t[:, h:h + hn][:, :, None].to_broadcast([128, hn, 128]),
                    in1=iota_sb[:][:, None, :].to_broadcast([128, hn, 128]),
                    op=mybir.AluOpType.is_equal,
                )
                oh_tiles.append(oh)
            # scatter in 64-slot half-chunks (merging 128-aligned pairs that
            # share a cell into one full-K matmul)
            nhc = ns // 64
            hc = 0
            while hc < nhc:
                ghc = s0 // 64 + hc
                ci_cell = int(hc_cell[ghc])
                if ci_cell < 0:
                    hc += 1          # group-alignment filler slots
                    continue
                g_, w_, hc0, nhc_cell = cells[ci_cell]
                pair = (hc % 2 == 0 and hc + 1 < nhc
                        and int(hc_cell[ghc + 1]) == ci_cell)
                lc = hc // 2
                if ci_cell not in cell_psum:
                    cell_psum[ci_cell] = psp.tile(
                        [128, used], F32, tag="cps", name=f"cps{ci_cell}")
                    cell_done[ci_cell] = 0
                first = cell_done[ci_cell] == 0
                take = 2 if pair else 1
                cell_done[ci_cell] += take
                last = cell_done[ci_cell] == nhc_cell
                if pair:
                    lhs = oh_tiles[lc // 32][:, lc % 32, :]
                    rhs = msg_t[:, lc, :used]
                else:
                    p0 = (hc % 2) * 64
                    lhs = oh_tiles[lc // 32][p0:p0 + 64, lc % 32, :]
                    rhs = msg_t[p0:p0 + 64, lc, :used]
                nc.tensor.matmul(
                    out=cell_psum[ci_cell][:], lhsT=lhs, rhs=rhs,
                    start=first, stop=last,
                )
                if last:
                    nc.vector.tensor_tensor(
                        out=acc[:, w_, :], in0=acc[:, w_, :],
                        in1=cell_psum[ci_cell][:], op=mybir.AluOpType.add,
                    )
                    del cell_psum[ci_cell]
                hc += take


def build_program(cfg, meta):
    N, F1, F2 = cfg["N"], cfg["F1"], cfg["F2"]
    SHARD, NW, SHARD_PAD = cfg["SHARD"], cfg["NW"], cfg["SHARD_PAD"]
    ROWS, GW, XW = cfg["ROWS"], cfg["GW"], cfg["XW"]
    L1, L2 = meta["L1"], meta["L2"]
    GWIN = GW // 128                   # windows per group

    nc = bacc.Bacc(None, target_bir_lowering=False, debug=False,
                   num_swdge_queues=NQUEUES,
                   dynamic_dma_scratch_size=DMA_SCRATCH)
    xT_d = nc.dram_tensor("xT", [F1, ROWS], F16, kind="ExternalInput")
    xTo_d = nc.dram_tensor("xTo", [F1, SHARD_PAD], F16, kind="ExternalInput")
    W1_d = nc.dram_tensor("W1", [F1, F1], F16, kind="ExternalInput")
    W2_d = nc.dram_tensor("W2", [F1, F2], F16, kind="ExternalInput")
    b1b_d = nc.dram_tensor("b1b", [128, F1], F32, kind="ExternalInput")
    b2b_d = nc.dram_tensor("b2b", [128, F2], F32, kind="ExternalInput")
    dinvO_d = nc.dram_tensor("dinvO", [128, NW], F32, kind="ExternalInput")
    iota_d = nc.dram_tensor("iota", [128, 128], F16, kind="ExternalInput")
    ident_d = nc.dram_tensor("ident", [128, 128], F16, kind="ExternalInput")
    idx1_d = nc.dram_tensor("idx1", [128, L1["S"] // 16], I16, kind="ExternalInput")
    dst1_d = nc.dram_tensor("dst1", [128, L1["S"] // 128], F16, kind="ExternalInput")
    idx2_d = nc.dram_tensor("idx2", [128, L2["S"] // 16], I16, kind="ExternalInput")
    dst2_d = nc.dram_tensor("dst2", [128, L2["S"] // 128], F16, kind="ExternalInput")
    out_d = nc.dram_tensor("out", [SHARD_PAD, F2], F32, kind="ExternalOutput")

    # per-group layer-1 tables so group-g gathers can start while phase A
    # still builds later groups
    g1_ds = [nc.dram_tensor(f"g1_tbl{g}", [GW, F1], F16) for g in range(NGROUPS)]
    g2s_d = nc.dram_tensor("g2_shard", [SHARD_PAD, 128], F16)
    g2f_d = nc.dram_tensor("g2_full", [ROWS, 128], F16, addr_space="Shared")
    g2f_groups = [g2f_d[g * GW:(g + 1) * GW, :] for g in range(NGROUPS)]

    with tile.TileContext(nc) as tc:
        with tc.tile_pool(name="persist", bufs=1) as pp:
            w1_sb = pp.tile([F1, F1], F16)
            nc.sync.dma_start(out=w1_sb[:], in_=W1_d[:, :])
            w2_sb = pp.tile([F1, F2], F16)
            nc.sync.dma_start(out=w2_sb[:], in_=W2_d[:, :])
            b1_sb = pp.tile([128, F1], F32)
            nc.sync.dma_start(out=b1_sb[:], in_=b1b_d[:, :])
            b2_sb = pp.tile([128, F2], F32)
            nc.sync.dma_start(out=b2_sb[:], in_=b2b_d[:, :])
            dinvO_sb = pp.tile([128, NW], F32)
            nc.sync.dma_start(out=dinvO_sb[:], in_=dinvO_d[:, :])
            iota_sb = pp.tile([128, 128], F16)
            nc.sync.dma_start(out=iota_sb[:], in_=iota_d[:, :])
            ident_sb = pp.tile([128, 128], F16)
            nc.sync.dma_start(out=ident_sb[:], in_=ident_d[:, :])

            def emit_phase_a_group(g, pa, pap):
                for v0 in range(0, GWIN, PHA_WIN):
                    nwv = min(PHA_WIN, GWIN - v0)
                    col0 = g * GW + v0 * 128
                    xt = pa.tile([128, PHA_WIN * 128], F16, tag="xt")
                    nc.sync.dma_start(
                        out=xt[:, :nwv * 128],
                        in_=xT_d[:, col0:col0 + nwv * 128])
                    hp = pap.tile([128, PHA_WIN * 128], F32, tag="hp")
                    for j in range(nwv):
                        nc.tensor.matmul(
                            out=hp[:, j * F1:(j + 1) * F1],
                            lhsT=xt[:, j * 128:(j + 1) * 128],
                            rhs=w1_sb[:], start=True, stop=True)
                    g1t = pa.tile([128, PHA_WIN, F1], F16, tag="g1t")
                    nc.scalar.activation(
                        out=g1t[:, :nwv, :],
                        in_=hp[:, :nwv * F1].rearrange("p (w f) -> p w f", w=nwv),
                        func=mybir.ActivationFunctionType.Copy)
                    nc.sync.dma_start(
                        out=g1_ds[g][v0 * 128:(v0 + nwv) * 128, :]
                            .rearrange("(w p) f -> p w f", p=128),
                        in_=g1t[:, :nwv, :])

            # ---- Phases A+B: layer-1 table build and aggregation ----
            with tc.tile_pool(name="acc2p", bufs=1) as accp2:
                acc2 = accp2.tile([128, NW, F2], F32)
                nc.vector.memset(acc2[:], 0.0)

                with tc.tile_pool(name="acc1", bufs=1) as accp:
                    acc = accp.tile([128, NW, F1], F32)
                    nc.vector.memset(acc[:], 0.0)

                    with (
                        tc.tile_pool(name="agg_sb1", bufs=MSG_BUFS) as sb2,
                        tc.tile_pool(name="agg_ix1", bufs=8) as ixp,
                        tc.tile_pool(name="agg_oh1", bufs=2) as ohp,
                        tc.tile_pool(name="agg_ps1", bufs=4, space="PSUM") as psp,
                    ):
                        tbl1 = dict(idx=idx1_d, dst=dst1_d, tbl=g1_ds)
                        pools1 = (sb2, ixp, ohp, psp)

                        # prefetch the first calls' idx/dst BEFORE phase A is
                        # emitted: otherwise these DMAs queue behind all of
                        # phase A's loads/stores on the HWDGE engines and the
                        # first gather idles ~300us past table-0 readiness
                        preloaded = {}
                        for ci, (g, s0, ns) in enumerate(L1["calls"][:8]):
                            idx_t = ixp.tile([128, CALL // 16], I16, tag="idx")
                            nc.scalar.dma_start(
                                out=idx_t[:, :ns // 16],
                                in_=idx1_d[:, s0 // 16:(s0 + ns) // 16])
                            dst_t = ixp.tile([128, CALL // 128], F16, tag="dst")
                            nc.scalar.dma_start(
                                out=dst_t[:, :ns // 128],
                                in_=dst1_d[:, s0 // 128:(s0 + ns) // 128])
                            preloaded[ci] = (idx_t, dst_t)

                        # phase A: g1 table (full, redundant per core, fp16)
                        with (
                            tc.tile_pool(name="pha_sb", bufs=3) as pa,
                            tc.tile_pool(name="pha_ps", bufs=2, space="PSUM") as pap,
                        ):
                            for g in range(NGROUPS):
                                emit_phase_a_group(g, pa, pap)

                        # self-loop term: acc[c] += g1[c] computed from the
                        # own-shard x slice (no gather descriptors needed)
                        with (
                            tc.tile_pool(name="self1", bufs=2) as sfp,
                            tc.tile_pool(name="self1_ps", bufs=2, space="PSUM") as sfps,
                        ):
                            for w in range(NW):
                                xo = sfp.tile([128, 128], F16, tag="xo")
                                nc.sync.dma_start(
                                    out=xo[:], in_=xTo_d[:, w * 128:(w + 1) * 128])
                                hps = sfps.tile([128, F1], F32, tag="hps")
                                nc.tensor.matmul(out=hps[:], lhsT=xo[:],
                                                 rhs=w1_sb[:],
                                                 start=True, stop=True)
                                nc.vector.tensor_tensor(
                                    out=acc[:, w, :], in0=acc[:, w, :],
                                    in1=hps[:], op=mybir.AluOpType.add)

                        for g in range(NGROUPS):
                            _emit_agg_group(nc, L1, tbl1, F1, F1, acc,
                                            iota_sb, pools1, g,
                                            preloaded=preloaded)

                    # open phase-D pools and prefetch its first calls'
                    # idx/dst now, so layer-2 desc-gen starts the moment the
                    # collective lands instead of queueing behind phase C
                    _es = contextlib.ExitStack()
                    sb2b = _es.enter_context(
                        tc.tile_pool(name="agg_sb2", bufs=MSG_BUFS))
                    ixpb = _es.enter_context(tc.tile_pool(name="agg_ix2", bufs=8))
                    ohpb = _es.enter_context(tc.tile_pool(name="agg_oh2", bufs=2))
                    pspb = _es.enter_context(
                        tc.tile_pool(name="agg_ps2", bufs=4, space="PSUM"))
                    preloaded2 = {}
                    for ci, (g, s0, ns) in enumerate(L2["calls"][:8]):
                        idx_t = ixpb.tile([128, CALL // 16], I16, tag="idx")
                        nc.scalar.dma_start(
                            out=idx_t[:, :ns // 16],
                            in_=idx2_d[:, s0 // 16:(s0 + ns) // 16])
                        dst_t = ixpb.tile([128, CALL // 128], F16, tag="dst")
                        nc.scalar.dma_start(
                            out=dst_t[:, :ns // 128],
                            in_=dst2_d[:, s0 // 128:(s0 + ns) // 128])
                        preloaded2[ci] = (idx_t, dst_t)

                    # ---- Phase C: layer-1 finalize + g2 shard ----
                    with (
                        tc.tile_pool(name="fin1", bufs=3) as fp,
                        tc.tile_pool(name="fin1_ps", bufs=2, space="PSUM") as fpp,
                        tc.tile_pool(name="fin1_ps2", bufs=2, space="PSUM") as fpp2,
                    ):
                        for w in range(NW):
                            t = fp.tile([128, F1], F32, tag="t")
                            nc.vector.tensor_scalar_mul(
                                out=t[:], in0=acc[:, w, :],
                                scalar1=dinvO_sb[:, w:w + 1])
                            nc.vector.tensor_tensor(
                                out=t[:], in0=t[:], in1=b1_sb[:],
                                op=mybir.AluOpType.add)
                            z = fp.tile([128, F1], F32, tag="z")
                            nc.scalar.activation(
                                out=z[:], in_=t[:],
                                func=mybir.ActivationFunctionType.Relu)
                            z2 = fp.tile([128, F1], F16, tag="z2")
                            nc.vector.tensor_scalar_mul(
                                out=z2[:], in0=z[:],
                                scalar1=dinvO_sb[:, w:w + 1])
                            tp = fpp.tile([128, 128], F16, tag="tp")
                            nc.tensor.transpose(out=tp[:], in_=z2[:],
                                                identity=ident_sb[:])
                            zT = fp.tile([128, F1], F16, tag="zT")
                            nc.vector.tensor_copy(out=zT[:], in_=tp[:])
                            h2 = fpp2.tile([128, F2], F32, tag="h2")
                            nc.tensor.matmul(out=h2[:], lhsT=zT[:], rhs=w2_sb[:],
                                             start=True, stop=True)
                            # layer-2 self-loop term (no gather descriptors)
                            nc.vector.tensor_tensor(
                                out=acc2[:, w, :], in0=acc2[:, w, :], in1=h2[:],
                                op=mybir.AluOpType.add)
                            g2t = fp.tile([128, 128], F16, tag="g2t")
                            nc.scalar.activation(
                                out=g2t[:, :F2], in_=h2[:],
                                func=mybir.ActivationFunctionType.Copy)
                            nc.sync.dma_start(
                                out=g2s_d[w * 128:(w + 1) * 128, :], in_=g2t[:])

                    # ---- AllGather g2 ----
                    nc.gpsimd.collective_compute(
                        "AllGather", mybir.AluOpType.bypass,
                        replica_groups=[list(range(NCORES))],
                        ins=[g2s_d.ap().opt()], outs=[g2f_d.ap().opt()])

                    # ---- Phase D: layer-2 aggregation ----
                    tbl2 = dict(idx=idx2_d, dst=dst2_d, tbl=g2f_groups)
                    pools2 = (sb2b, ixpb, ohpb, pspb)
                    for g in range(NGROUPS):
                        _emit_agg_group(nc, L2, tbl2, 128, F2, acc2,
                                        iota_sb, pools2, g,
                                        preloaded=preloaded2)
                    _es.close()

                # ---- Phase E: layer-2 finalize ----
                with tc.tile_pool(name="fin2", bufs=3) as fp2:
                    for w in range(NW):
                        o = fp2.tile([128, F2], F32, tag="o")
                        nc.vector.tensor_scalar_mul(
                            out=o[:], in0=acc2[:, w, :],
                            scalar1=dinvO_sb[:, w:w + 1])
                        nc.vector.tensor_tensor(
                            out=o[:], in0=o[:], in1=b2_sb[:],
                            op=mybir.AluOpType.add)
                        nc.sync.dma_start(
                            out=out_d[w * 128:(w + 1) * 128, :], in_=o[:])

    nc.finalize()
    return nc


def _run(x, edge_index, W1, b1, W2, b2, n_nodes, trace=False):
    cfg = _cfg(n_nodes, int(W1.shape[1]), int(W2.shape[1]))
    in_maps, meta = _prep(x, edge_index, W1, b1, W2, b2, cfg)
    nc = build_program(cfg, meta)
    res = bass_utils.run_bass_kernel_spmd(
        nc, in_maps, core_ids=list(range(NCORES)), trace=trace)
    SHARD, SHARD_PAD, F2 = cfg["SHARD"], cfg["SHARD_PAD"], cfg["F2"]
    out = np.concatenate(
        [res.results[k]["out"][:SHARD] for k in range(NCORES)], axis=0)
    return out[:n_nodes], res


def kernel(x, edge_index, W1, b1, W2, b2):
    x = np.asarray(x)
    out, _ = _run(np.asarray(x, np.float32), np.asarray(edge_index),
                  np.asarray(W1, np.float32), np.asarray(b1, np.float32),
                  np.asarray(W2, np.float32), np.asarray(b2, np.float32),
                  n_nodes=x.shape[0])
    return out.astype(np.float32)
